# revision 7
# baseline (speedup 1.0000x reference)
"""Spiking transformer block (nn_Block_22170621182450) on 8 trn2 NeuronCores.

Data-parallel over B (2 batch elems/core). Channel-major on-chip layout
[C_out, tokens]; tokens are t-major so LIF time slabs are contiguous.
BN statistics are globalized with tiny AllReduces (sum, sumsq per channel).

v2 precision/throughput plan (validated by numpy flip-simulation, sim.py):
  - q/k/v linears: single-pass float32r (PE rounds f32 operands to the
    11-bit f32r grid internally; raw f32 fed via bitcast).
  - p: fp8e4m3 DoubleRow (0.5 cyc/row), weight hi/lo split pre-scaled by
    2^7 so the lo part stays in e4m3's normal range; descaled 1/128 in the
    PSUM->SBUF copy. Activations are binary spikes (exact in fp8).
  - f1: 2-pass float32r weight hi/lo split (w_hi*x + w_lo*x), x fed raw.
  - f2: fp8e4m3 DoubleRow 3-way weight split, pre-scaled 2^7.
  - attention: bf16 (binary spikes / small integers -> exact).
Sim rel-err of this config vs exact reference: 1.24e-2 (gate 2e-2).
Stats: one [128, 4*SLAB] PSUM tile per output-channel tile; a single
activation Copy (with accum) extracts h + per-channel sums, a single
in-place Square (accum) yields sumsq. Linear biases dropped (BN absorbs).
"""

import os
import sys

for p in ("/opt/trn_rl_repo", "/root/.axon_site", "/root/.axon_site/_ro/trn_rl_repo",
          "/root/.axon_site/_ro/pypackages"):
    if os.path.isdir(p) and p not in sys.path:
        sys.path.append(p)

import numpy as np
import ml_dtypes

from contextlib import ExitStack
import concourse.bass as bass
import concourse.bacc as bacc
import concourse.tile as tile
from concourse import mybir
from concourse import bass_utils
from concourse.dve_ops import TENSOR_MASK
from concourse.masks import make_identity

F32 = mybir.dt.float32
F32R = mybir.dt.float32r
BF16 = mybir.dt.bfloat16
FP8 = mybir.dt.float8e4
AX = mybir.AluOpType
AF = mybir.ActivationFunctionType
DR = mybir.MatmulPerfMode.DoubleRow

T, B, N, C, H = 4, 16, 256, 512, 8
HID = 2048
NCORES = 8
BL = B // NCORES            # 2 batch elems per core
S = T * BL * N              # 2048 tokens per core
SLAB = BL * N               # 512 tokens per time step
S_TOT = T * B * N           # 16384 tokens globally (BN population)
CT_C = C // 128             # 4 channel tiles for C
CT_H = HID // 128           # 16 channel tiles for HID
EPS = 1e-5
SCALE = 0.125
P = 128
WSC = 128.0                 # fp8 weight pre-scale (2^7)

_CACHE = {}
NO_COLL = os.environ.get("KERNEL_NO_COLL", "0") == "1"
F1_PASSES = int(os.environ.get("KERNEL_F1_PASSES", "2"))


def _round_mant(x, m=11):
    """Round fp32 to m explicit mantissa bits (float32r grid)."""
    x = np.ascontiguousarray(x, np.float32)
    b = x.view(np.uint32).astype(np.uint64)
    shift = 23 - m
    add = np.uint64(1 << (shift - 1))
    mask = np.uint64(~((1 << shift) - 1) & 0xFFFFFFFF)
    return ((b + add) & mask).astype(np.uint32).view(np.float32)


def _f32r_split(x):
    hi = _round_mant(x, 11)
    lo = _round_mant(x.astype(np.float32) - hi, 11)
    return hi, lo


def _fp8_dr_pack(W, nsplit):
    """W [K, M] f32 -> nsplit DoubleRow-packed fp8 arrays [K//2, 2*M]:
    row g*128+p, col i*M+m  =  fp8(W*WSC residual_i)[g*256 + i*128 + p, m]."""
    K, M = W.shape
    acc = np.ascontiguousarray(W, np.float32) * np.float32(WSC)
    packed = []
    for _ in range(nsplit):
        w8 = acc.astype(ml_dtypes.float8_e4m3)
        acc = acc - w8.astype(np.float32)
        pk = w8.reshape(K // 256, 2, 128, M).transpose(0, 2, 1, 3)
        packed.append(np.ascontiguousarray(pk.reshape(K // 2, 2 * M)))
    return packed


def _pack_ch(v, n_ct):
    """[n_ct*128] channel vector -> [128, n_ct] (channel%128 on partitions)."""
    return np.ascontiguousarray(np.asarray(v, np.float32).reshape(n_ct, P).T)


def _emit(nc, tc, tens, rep):
    """Emit one full block-forward over the DRAM tensors in `tens`.
    rep > 0 replicas read x from outT (garbage values, timing only)."""
    xt_d = tens["outT"] if rep > 0 else tens["xt"]
    w_lin = {nm: tens["w_" + nm] for nm in ("q", "k", "v")}
    RG = [list(range(NCORES))]

    with ExitStack() as _es:
        constp = _es.enter_context(tc.tile_pool(name=f"const{rep}", bufs=1))
        gbep = _es.enter_context(tc.tile_pool(name=f"gbep{rep}", bufs=1))
        xp = _es.enter_context(tc.tile_pool(name=f"xsplit{rep}", bufs=4))
        mp = _es.enter_context(tc.tile_pool(name=f"mstate{rep}", bufs=2))
        statp = _es.enter_context(tc.tile_pool(name=f"stats{rep}", bufs=4))
        bnp = _es.enter_context(tc.tile_pool(name=f"bnconst{rep}", bufs=4))
        dramp = _es.enter_context(tc.tile_pool(name=f"dram{rep}", bufs=2, space="DRAM"))

        eps_t = constp.tile([P, 1], F32, tag="eps")
        nc.vector.memset(eps_t[:], EPS)
        ident = constp.tile([P, P], BF16, tag="ident")
        make_identity(nc, ident[:])

        gbe_sb = {}
        for nm in ("q", "k", "v", "p", "f1", "f2"):
            n_ct = CT_H if nm == "f1" else CT_C
            gt = gbep.tile([P, n_ct], F32, tag=f"g_{nm}")
            bt = gbep.tile([P, n_ct], F32, tag=f"b_{nm}")
            nc.sync.dma_start(out=gt[:], in_=tens[f"{nm}_gp"])
            nc.sync.dma_start(out=bt[:], in_=tens[f"{nm}_bp"])
            gbe_sb[nm] = (gt, bt)

        # x, channel-major [C, S] as 4 tiles of [128, S] f32r
        xs = []
        for ct in range(CT_C):
            tx = xp.tile([P, S], F32R, tag="xs")
            nc.sync.dma_start(out=tx[:], in_=xt_d[ct * P:(ct + 1) * P, :])
            xs.append(tx)

        # ---------- helpers ----------
        def bn_affine(gstats, n_ct, g_sl, b_sl):
            """gstats [128, 2*n_ct] = [sums | sumsqs] (global).
            Returns (a_sc, c_sc) [128, 4*n_ct]: per-t-scaled affine."""
            mean = bnp.tile([P, n_ct], F32, tag="mean")
            var = bnp.tile([P, n_ct], F32, tag="var")
            tmpb = bnp.tile([P, n_ct], F32, tag="btmp")
            nc.vector.tensor_scalar(mean[:], gstats[:, 0:n_ct],
                                    1.0 / S_TOT, None, AX.mult)
            nc.vector.tensor_scalar(var[:], gstats[:, n_ct:2 * n_ct],
                                    1.0 / S_TOT, None, AX.mult)
            nc.vector.tensor_mul(tmpb[:], mean[:], mean[:])
            nc.vector.tensor_tensor(out=var[:], in0=var[:], in1=tmpb[:],
                                    op=AX.subtract)
            nc.scalar.activation(var[:], var[:], AF.Sqrt, bias=eps_t[:])
            nc.vector.reciprocal(var[:], var[:])
            a0 = bnp.tile([P, n_ct], F32, tag="a0")
            c0 = bnp.tile([P, n_ct], F32, tag="c0")
            nc.vector.tensor_mul(a0[:], var[:], g_sl)
            nc.vector.tensor_mul(tmpb[:], mean[:], a0[:])
            nc.vector.tensor_tensor(out=c0[:], in0=b_sl, in1=tmpb[:],
                                    op=AX.subtract)
            a_sc = bnp.tile([P, 4 * n_ct], F32, tag="asc")
            c_sc = bnp.tile([P, 4 * n_ct], F32, tag="csc")
            for t in range(T):
                s = float(2.0 ** (t - 1))
                nc.vector.tensor_scalar(a_sc[:, t * n_ct:(t + 1) * n_ct],
                                        a0[:], s, None, AX.mult)
                nc.vector.tensor_scalar(c_sc[:, t * n_ct:(t + 1) * n_ct],
                                        c0[:], s, None, AX.mult)
            return a_sc, c_sc

        def lif_tile(h_t, ct, n_ct, a_sc, c_sc, spk_writer):
            """LIF over the 4 time slabs of h_t [128, S].
            a_sc/c_sc None -> raw input, scale 2^(t-1) (o-lif).
            spk_writer(t, m_ap) emits the spike tensor for slab t."""
            m = mp.tile([P, SLAB], F32, tag="m")
            for t in range(T):
                sl = h_t[:, t * SLAB:(t + 1) * SLAB]
                thr = float(2.0 ** t)
                if a_sc is None:
                    sa = float(2.0 ** (t - 1))
                    if t == 0:
                        nc.vector.tensor_scalar(m[:], sl, sa, None, AX.mult)
                    else:
                        nc.vector.affine_then_add(m[:], sl, m[:], sa, 0.0)
                else:
                    sa = a_sc[:, t * n_ct + ct:t * n_ct + ct + 1]
                    sc = c_sc[:, t * n_ct + ct:t * n_ct + ct + 1]
                    if t == 0:
                        nc.vector.tensor_scalar(m[:], sl, sa, sc,
                                                AX.mult, AX.add)
                    else:
                        nc.vector.affine_then_add(m[:], sl, m[:], sa, sc)
                spk_writer(t, m[:])
                if t < T - 1:
                    nc.vector._custom_dve(TENSOR_MASK, out=m[:], in0=m[:],
                                          in1=m[:], s0=thr, imm2=0.0)

        def allreduce_stats(arin_sb, width):
            ain = dramp.tile([P, width], F32, tag="arin")
            aout = dramp.tile([P, width], F32, tag="arout", addr_space="Shared")
            nc.sync.dma_start(out=ain[:], in_=arin_sb[:])
            if NO_COLL:
                nc.sync.dma_start(out=aout[:], in_=ain[:])
            else:
                nc.gpsimd.collective_compute(
                    "AllReduce", AX.add, replica_groups=RG,
                    ins=[ain.opt()], outs=[aout.opt()])
            g = statp.tile([P, width], F32, tag="gstats")
            nc.sync.dma_start(out=g[:], in_=aout[:])
            return g

        def mm_layer(n_ct, matmuls_fn, psum_pool, h_pool, descale=1.0):
            """For each output-channel tile ct: run matmuls_fn(ct, ts, ps_slab)
            filling a [P, 4*SLAB] PSUM tile, then one Copy (h + sum accum)
            and one in-place Square (sumsq accum). Returns (h_tiles, arin)."""
            arin = statp.tile([P, 2 * n_ct], F32, tag="arin_sb")
            h_tiles = []
            for ct in range(n_ct):
                ps = psum_pool.tile([P, 4 * SLAB], F32, tag="ps")
                for ts in range(T):
                    matmuls_fn(ct, ts, ps[:, ts * SLAB:(ts + 1) * SLAB])
                h_t = h_pool.tile([P, S], F32, tag="h")
                h_tiles.append(h_t)
                nc.scalar.activation(
                    h_t[:], ps[:], AF.Copy, scale=descale,
                    accum_out=arin[:, ct:ct + 1])
                nc.scalar.activation(
                    ps[:], ps[:], AF.Square, scale=descale,
                    accum_out=arin[:, n_ct + ct:n_ct + ct + 1])
            return h_tiles, arin

        # =======================================================
        # Phases 1-4: qkv (1-pass f32r) / attention / o-lif / p
        # =======================================================
        with ExitStack() as _es1:
            wqp = _es1.enter_context(tc.tile_pool(name=f"wqkv{rep}", bufs=4))
            hp = _es1.enter_context(tc.tile_pool(name=f"hpool{rep}", bufs=6))
            spkp = _es1.enter_context(tc.tile_pool(name=f"spkbf{rep}", bufs=12))
            opairp = _es1.enter_context(tc.tile_pool(name=f"opair{rep}", bufs=2))

            spk_qkv = {}
            with tc.tile_pool(name=f"psqkv{rep}", bufs=2, space="PSUM") as psmm:
                for nm in ("q", "k", "v"):
                    wsb = []
                    for kc in range(CT_C):
                        w_sb = wqp.tile([P, C], F32R, tag="wq")
                        nc.sync.dma_start(
                            out=w_sb[:], in_=w_lin[nm][kc * P:(kc + 1) * P, :])
                        wsb.append(w_sb)

                    def mm_qkv(ct, ts, ps_slab, wsb=wsb):
                        for kc in range(CT_C):
                            nc.tensor.matmul(
                                ps_slab,
                                wsb[kc][:, ct * P:(ct + 1) * P],
                                xs[kc][:, ts * SLAB:(ts + 1) * SLAB],
                                start=(kc == 0), stop=(kc == CT_C - 1))

                    h_tiles, arin = mm_layer(CT_C, mm_qkv, psmm, hp)
                    gst = allreduce_stats(arin, 2 * CT_C)
                    g_t, b_t = gbe_sb[nm]
                    a_sc, c_sc = bn_affine(gst, CT_C, g_t[:], b_t[:])
                    tiles = []
                    for ct in range(CT_C):
                        s_t = spkp.tile([P, S], BF16, tag="spk")
                        tiles.append(s_t)

                        def wr(t, m_ap, s_t=s_t):
                            nc.vector.tensor_scalar(
                                s_t[:, t * SLAB:(t + 1) * SLAB], m_ap,
                                float(2.0 ** t), None, AX.is_gt)

                        lif_tile(h_tiles[ct], ct, CT_C, a_sc, c_sc, wr)
                    spk_qkv[nm] = tiles

            # ---- attention (o_tiles f32; spikes to fp8 DoubleRow pairs) ----
            o_tiles = [hp.tile([P, S], F32, tag="h", name="o_t")
                       for _ in range(CT_C)]
            with ExitStack() as _es2:
                atp = _es2.enter_context(tc.tile_pool(name=f"attn{rep}", bufs=3))
                kvp = _es2.enter_context(tc.tile_pool(name=f"kvp{rep}", bufs=4))
                pst = _es2.enter_context(tc.tile_pool(name=f"pst{rep}", bufs=2, space="PSUM"))
                pskv = _es2.enter_context(tc.tile_pool(name=f"pskv{rep}", bufs=2, space="PSUM"))
                pso = _es2.enter_context(tc.tile_pool(name=f"pso{rep}", bufs=2, space="PSUM"))
                for tb in range(T * BL):
                    base = tb * N
                    kT = [atp.tile([P, C], BF16, tag="kT", name="kT")
                          for _ in range(2)]
                    vT = [atp.tile([P, C], BF16, tag="vT", name="vT")
                          for _ in range(2)]
                    for ct in range(CT_C):
                        for hf in range(2):
                            for src, dst in ((spk_qkv["k"][ct], kT[hf]),
                                             (spk_qkv["v"][ct], vT[hf])):
                                pt = pst.tile([P, P], BF16, tag="pt")
                                nc.tensor.transpose(
                                    pt[:],
                                    src[:, base + hf * P:base + (hf + 1) * P],
                                    ident[:])
                                nc.scalar.copy(
                                    dst[:, ct * P:(ct + 1) * P], pt[:])
                    for ct in range(CT_C):
                        po = pso.tile([P, N], F32, tag="po")
                        for hf in range(2):
                            hd = ct * 2 + hf
                            cols = slice(hd * 64, (hd + 1) * 64)
                            prt = slice(hf * 64, (hf + 1) * 64)
                            pkv = pskv.tile([P, 64], F32, tag="pkv")
                            nc.tensor.matmul(pkv[prt, :], kT[0][:, cols],
                                             vT[0][:, cols],
                                             start=True, stop=False)
                            nc.tensor.matmul(pkv[prt, :], kT[1][:, cols],
                                             vT[1][:, cols],
                                             start=False, stop=True)
                            kv_sb = kvp.tile([P, 64], BF16, tag="kv")
                            nc.scalar.mul(kv_sb[prt, :], pkv[prt, :], SCALE)
                            nc.tensor.matmul(
                                po[prt, :], kv_sb[prt, :],
                                spk_qkv["q"][ct][prt, base:base + N],
                                start=True, stop=True)
                        nc.scalar.copy(
                            o_tiles[ct][:, base:base + N], po[:])

            # ---- o-lif: spikes -> fp8 pair-packed tiles ----
            o_pair = [opairp.tile([P, 2 * S], FP8, tag="opair", name="o_pair")
                      for _ in range(CT_C // 2)]
            for ct in range(CT_C):
                g, i = ct // 2, ct % 2

                def wr_o(t, m_ap, g=g, i=i):
                    nc.vector.tensor_scalar(
                        o_pair[g][:, i * S + t * SLAB:i * S + (t + 1) * SLAB],
                        m_ap, float(2.0 ** t), None, AX.is_gt)

                lif_tile(o_tiles[ct], ct, CT_C, None, None, wr_o)

            # ---- p projection: fp8 DoubleRow, 2-pass weight split ----
            with tc.tile_pool(name=f"psp{rep}", bufs=2, space="PSUM") as psp:
                wp_sb = {}
                for wi, key in ((0, "w_pa"), (1, "w_pb")):
                    for g in range(2):
                        w_sb = wqp.tile([P, 2 * C], FP8, tag="wp8")
                        nc.sync.dma_start(
                            out=w_sb[:], in_=tens[key][g * P:(g + 1) * P, :])
                        wp_sb[(wi, g)] = w_sb

                def mm_p(ct, ts, ps_slab):
                    first = True
                    for wi in range(2):
                        for g in range(2):
                            w3 = wp_sb[(wi, g)][:].rearrange(
                                "p (i m) -> p i m", i=2)
                            s3 = o_pair[g][:].rearrange(
                                "p (i s) -> p i s", i=2)
                            nc.tensor.matmul(
                                ps_slab,
                                w3[:, :, ct * P:(ct + 1) * P],
                                s3[:, :, ts * SLAB:(ts + 1) * SLAB],
                                start=first, stop=(wi == 1 and g == 1),
                                perf_mode=DR)
                            first = False

                hp_tiles, arin_p = mm_layer(CT_C, mm_p, psp, hp, descale=1.0 / WSC)
                gst_p = allreduce_stats(arin_p, 2 * CT_C)
                g_t, b_t = gbe_sb["p"]
                a_sc, c_sc = bn_affine(gst_p, CT_C, g_t[:], b_t[:])
                spk_p = []
                for ct in range(CT_C):
                    s_t = spkp.tile([P, S], BF16, tag="spk")
                    spk_p.append(s_t)

                    def wr_p(t, m_ap, s_t=s_t):
                        nc.vector.tensor_scalar(
                            s_t[:, t * SLAB:(t + 1) * SLAB], m_ap,
                            float(2.0 ** t), None, AX.is_gt)

                    lif_tile(hp_tiles[ct], ct, CT_C, a_sc, c_sc, wr_p)

            # ---- x_res = x + spk_p, in place in the x tiles ----
            xres = xs
            for ct in range(CT_C):
                nc.vector.tensor_tensor(out=xs[ct][:], in0=xs[ct][:],
                                        in1=spk_p[ct][:], op=AX.add)

        # =======================================================
        # Phase 6: f1 (2-pass f32r weight split), AllReduce per half
        # =======================================================
        with ExitStack() as _es3:
            sp8p = _es3.enter_context(tc.tile_pool(name=f"sp8{rep}", bufs=8))
            hp2 = _es3.enter_context(tc.tile_pool(name=f"hpool2{rep}", bufs=8))
            spk_f1p = [sp8p.tile([P, 2 * S], FP8, tag="spk8", name="spk_f1p")
                       for _ in range(CT_H // 2)]
            with ExitStack() as _es3b:
                wsp = _es3b.enter_context(tc.tile_pool(name=f"wsm{rep}", bufs=9))
                psm2 = _es3b.enter_context(
                    tc.tile_pool(name=f"psf1{rep}", bufs=2, space="PSUM"))
                assert F1_PASSES in (1, 2)
                for half in range(2):
                    wt = {}
                    for wi, w_d in ((0, tens["w_f1h"]),
                                    (1, tens["w_f1l"]))[:F1_PASSES]:
                        for kc in range(CT_C):
                            w_sb = wsp.tile([P, 8 * P], F32R, tag="wf1")
                            nc.sync.dma_start(
                                out=w_sb[:],
                                in_=w_d[kc * P:(kc + 1) * P,
                                        half * 8 * P:(half + 1) * 8 * P])
                            for cl in range(8):
                                wt[(wi, kc, cl)] = w_sb[:, cl * P:(cl + 1) * P]

                    def mm_f1(cl, ts, ps_slab, wt=wt):
                        first = True
                        for wi in range(F1_PASSES):
                            for kc in range(CT_C):
                                nc.tensor.matmul(
                                    ps_slab, wt[(wi, kc, cl)],
                                    xres[kc][:, ts * SLAB:(ts + 1) * SLAB],
                                    start=first,
                                    stop=(wi == F1_PASSES - 1 and
                                          kc == CT_C - 1))
                                first = False

                    h_tiles, arin_f = mm_layer(8, mm_f1, psm2, hp2)
                    gst_f = allreduce_stats(arin_f, 16)
                    g_t, b_t = gbe_sb["f1"]
                    a_sc, c_sc = bn_affine(
                        gst_f, 8, g_t[:, half * 8:half * 8 + 8],
                        b_t[:, half * 8:half * 8 + 8])
                    for cl in range(8):
                        ct = half * 8 + cl
                        g, i = ct // 2, ct % 2

                        def wr_f1(t, m_ap, g=g, i=i):
                            nc.vector.tensor_scalar(
                                spk_f1p[g][:, i * S + t * SLAB:
                                           i * S + (t + 1) * SLAB],
                                m_ap, float(2.0 ** t), None, AX.is_gt)

                        lif_tile(h_tiles[cl], cl, 8, a_sc, c_sc, wr_f1)

            # =====================================================
            # Phase 7: f2: fp8 DoubleRow, 3-way weight split
            # =====================================================
            with ExitStack() as _es4:
                wf2p = _es4.enter_context(tc.tile_pool(name=f"wf2p{rep}", bufs=24))
                psm3 = _es4.enter_context(
                    tc.tile_pool(name=f"psf2{rep}", bufs=2, space="PSUM"))
                wf2_sb = {}
                for wi, key in ((0, "w_f2a"), (1, "w_f2b"), (2, "w_f2c")):
                    for g in range(CT_H // 2):
                        w_sb = wf2p.tile([P, 2 * C], FP8, tag="wf2")
                        nc.sync.dma_start(
                            out=w_sb[:], in_=tens[key][g * P:(g + 1) * P, :])
                        wf2_sb[(wi, g)] = w_sb

                def mm_f2(ct, ts, ps_slab):
                    first = True
                    for wi in range(3):
                        for g in range(CT_H // 2):
                            w3 = wf2_sb[(wi, g)][:].rearrange(
                                "p (i m) -> p i m", i=2)
                            s3 = spk_f1p[g][:].rearrange(
                                "p (i s) -> p i s", i=2)
                            nc.tensor.matmul(
                                ps_slab,
                                w3[:, :, ct * P:(ct + 1) * P],
                                s3[:, :, ts * SLAB:(ts + 1) * SLAB],
                                start=first,
                                stop=(wi == 2 and g == CT_H // 2 - 1),
                                perf_mode=DR)
                            first = False

                h2_tiles, arin_2 = mm_layer(CT_C, mm_f2, psm3, hp2,
                                            descale=1.0 / WSC)
                gst_2 = allreduce_stats(arin_2, 2 * CT_C)
                g_t, b_t = gbe_sb["f2"]
                a_sc, c_sc = bn_affine(gst_2, CT_C, g_t[:], b_t[:])
                for ct in range(CT_C):
                    h_t = h2_tiles[ct]

                    def wr_f2(t, m_ap, h_t=h_t):
                        nc.vector.tensor_scalar(
                            h_t[:, t * SLAB:(t + 1) * SLAB], m_ap,
                            float(2.0 ** t), None, AX.is_gt)

                    lif_tile(h_t, ct, CT_C, a_sc, c_sc, wr_f2)

                # out = x_res + spk_f2
                for ct in range(CT_C):
                    tmp = hp2.tile([P, S], F32, tag="h", name="outadd")
                    nc.vector.tensor_tensor(out=tmp[:], in0=xres[ct][:],
                                            in1=h2_tiles[ct][:], op=AX.add)
                    nc.sync.dma_start(
                        out=tens["outT"][ct * P:(ct + 1) * P, :], in_=tmp[:])


def _build(replicas=1):
    nc = bacc.Bacc("TRN2", target_bir_lowering=False, debug=False,
                   num_devices=NCORES)

    def dt_in(name, shape, dtype):
        return nc.dram_tensor(name, list(shape), dtype,
                              kind="ExternalInput").ap()

    tens = {"xt": dt_in("xt", (C, S), F32R)}
    for nm in ("q", "k", "v"):
        tens["w_" + nm] = dt_in("w_" + nm, (C, C), F32R)
    tens["w_pa"] = dt_in("w_pa", (C // 2, 2 * C), FP8)
    tens["w_pb"] = dt_in("w_pb", (C // 2, 2 * C), FP8)
    tens["w_f1h"] = dt_in("w_f1h", (C, HID), F32R)
    tens["w_f1l"] = dt_in("w_f1l", (C, HID), F32R)
    tens["w_f2a"] = dt_in("w_f2a", (HID // 2, 2 * C), FP8)
    tens["w_f2b"] = dt_in("w_f2b", (HID // 2, 2 * C), FP8)
    tens["w_f2c"] = dt_in("w_f2c", (HID // 2, 2 * C), FP8)
    for nm in ("q", "k", "v", "p", "f1", "f2"):
        n_ct = CT_H if nm == "f1" else CT_C
        tens[f"{nm}_gp"] = dt_in(f"{nm}_gp", (P, n_ct), F32)
        tens[f"{nm}_bp"] = dt_in(f"{nm}_bp", (P, n_ct), F32)
    tens["outT"] = nc.dram_tensor("outT", [C, S], F32,
                                  kind="ExternalOutput").ap()

    with tile.TileContext(nc) as tc:
        for rep in range(replicas):
            _emit(nc, tc, tens, rep)

    nc.compile()
    return nc


def _get_nc():
    if "nc" not in _CACHE:
        _CACHE["nc"] = _build()
    return _CACHE["nc"]


def _make_in_maps(inputs):
    x = np.asarray(inputs["x"], np.float32)
    base = {}
    for nm in ("q", "k", "v"):
        base[f"w_{nm}"] = _round_mant(np.asarray(inputs[f"{nm}_W"],
                                                 np.float32))
    base["w_pa"], base["w_pb"] = _fp8_dr_pack(
        np.asarray(inputs["p_W"], np.float32), 2)
    base["w_f1h"], base["w_f1l"] = _f32r_split(np.asarray(inputs["f1_W"],
                                                          np.float32))
    base["w_f2a"], base["w_f2b"], base["w_f2c"] = _fp8_dr_pack(
        np.asarray(inputs["f2_W"], np.float32), 3)
    for nm in ("q", "k", "v", "p", "f1", "f2"):
        n_ct = CT_H if nm == "f1" else CT_C
        base[f"{nm}_gp"] = _pack_ch(inputs[f"{nm}_g"], n_ct)
        base[f"{nm}_bp"] = _pack_ch(inputs[f"{nm}_be"], n_ct)
    in_maps = []
    for c in range(NCORES):
        xsl = x[:, c * BL:(c + 1) * BL].reshape(S, C)
        m = dict(base)
        m["xt"] = _round_mant(np.ascontiguousarray(xsl.T))
        in_maps.append(m)
    return in_maps


def kernel(**inputs):
    in_maps = _make_in_maps(inputs)
    nc = _get_nc()
    res = bass_utils.run_bass_kernel_spmd(nc, in_maps,
                                          core_ids=list(range(NCORES)))
    _CACHE["last_results"] = res

    out = np.empty((T, B, N, C), np.float32)
    for c in range(NCORES):
        oc = np.asarray(res.results[c]["outT"])   # [C, S]
        out[:, c * BL:(c + 1) * BL] = oc.T.reshape(T, BL, N, C)
    return out


# revision 9
# speedup vs baseline: 1.2041x; 1.2041x over previous
"""Spiking transformer block (nn_Block_22170621182450) on 8 trn2 NeuronCores.

Data-parallel over B (2 batch elems/core). Channel-major on-chip layout
[C_out, tokens]; tokens are t-major so LIF time slabs are contiguous.
BN statistics are globalized with tiny AllReduces (sum, sumsq per channel).

v2 precision/throughput plan (validated by numpy flip-simulation, sim.py):
  - q/k/v linears: single-pass float32r (PE rounds f32 operands to the
    11-bit f32r grid internally; raw f32 fed via bitcast).
  - p: fp8e4m3 DoubleRow (0.5 cyc/row), weight hi/lo split pre-scaled by
    2^7 so the lo part stays in e4m3's normal range; descaled 1/128 in the
    PSUM->SBUF copy. Activations are binary spikes (exact in fp8).
  - f1: 2-pass float32r weight hi/lo split (w_hi*x + w_lo*x), x fed raw.
  - f2: fp8e4m3 DoubleRow 3-way weight split, pre-scaled 2^7.
  - attention: bf16 (binary spikes / small integers -> exact).
Sim rel-err of this config vs exact reference: 1.24e-2 (gate 2e-2).
Stats: one [128, 4*SLAB] PSUM tile per output-channel tile; a single
activation Copy (with accum) extracts h + per-channel sums, a single
in-place Square (accum) yields sumsq. Linear biases dropped (BN absorbs).
"""

import os
import sys

for p in ("/opt/trn_rl_repo", "/root/.axon_site", "/root/.axon_site/_ro/trn_rl_repo",
          "/root/.axon_site/_ro/pypackages"):
    if os.path.isdir(p) and p not in sys.path:
        sys.path.append(p)

import numpy as np
import ml_dtypes

from contextlib import ExitStack
import concourse.bass as bass
import concourse.bacc as bacc
import concourse.tile as tile
from concourse import mybir
from concourse import bass_utils
from concourse.dve_ops import TENSOR_MASK
from concourse.masks import make_identity

F32 = mybir.dt.float32
F32R = mybir.dt.float32r
BF16 = mybir.dt.bfloat16
FP8 = mybir.dt.float8e4
AX = mybir.AluOpType
AF = mybir.ActivationFunctionType
DR = mybir.MatmulPerfMode.DoubleRow

T, B, N, C, H = 4, 16, 256, 512, 8
HID = 2048
NCORES = 8
BL = B // NCORES            # 2 batch elems per core
S = T * BL * N              # 2048 tokens per core
SLAB = BL * N               # 512 tokens per time step
S_TOT = T * B * N           # 16384 tokens globally (BN population)
CT_C = C // 128             # 4 channel tiles for C
CT_H = HID // 128           # 16 channel tiles for HID
EPS = 1e-5
SCALE = 0.125
P = 128
WSC = 128.0                 # fp8 weight pre-scale (2^7)

_CACHE = {}
NO_COLL = os.environ.get("KERNEL_NO_COLL", "0") == "1"
F1_PASSES = int(os.environ.get("KERNEL_F1_PASSES", "2"))


def _round_mant(x, m=11):
    """Round fp32 to m explicit mantissa bits (float32r grid)."""
    x = np.ascontiguousarray(x, np.float32)
    b = x.view(np.uint32).astype(np.uint64)
    shift = 23 - m
    add = np.uint64(1 << (shift - 1))
    mask = np.uint64(~((1 << shift) - 1) & 0xFFFFFFFF)
    return ((b + add) & mask).astype(np.uint32).view(np.float32)


def _f32r_split(x):
    hi = _round_mant(x, 11)
    lo = _round_mant(x.astype(np.float32) - hi, 11)
    return hi, lo


def _fp8_dr_pack(W, nsplit):
    """W [K, M] f32 -> nsplit DoubleRow-packed fp8 arrays [K//2, 2*M]:
    row g*128+p, col i*M+m  =  fp8(W*WSC residual_i)[g*256 + i*128 + p, m]."""
    K, M = W.shape
    acc = np.ascontiguousarray(W, np.float32) * np.float32(WSC)
    packed = []
    for _ in range(nsplit):
        w8 = acc.astype(ml_dtypes.float8_e4m3)
        acc = acc - w8.astype(np.float32)
        pk = w8.reshape(K // 256, 2, 128, M).transpose(0, 2, 1, 3)
        packed.append(np.ascontiguousarray(pk.reshape(K // 2, 2 * M)))
    return packed


def _pack_ch(v, n_ct):
    """[n_ct*128] channel vector -> [128, n_ct] (channel%128 on partitions)."""
    return np.ascontiguousarray(np.asarray(v, np.float32).reshape(n_ct, P).T)


def _emit(nc, tc, tens, rep):
    """Emit one full block-forward over the DRAM tensors in `tens`.
    rep > 0 replicas read x from outT (garbage values, timing only)."""
    xt_d = tens["outT"].bitcast(F32R) if rep > 0 else tens["xt"]
    w_lin = {nm: tens["w_" + nm] for nm in ("q", "k", "v")}
    RG = [list(range(NCORES))]

    with ExitStack() as _es:
        constp = _es.enter_context(tc.tile_pool(name=f"const{rep}", bufs=1))
        gbep = _es.enter_context(tc.tile_pool(name=f"gbep{rep}", bufs=1))
        xp = _es.enter_context(tc.tile_pool(name=f"xsplit{rep}", bufs=4))
        mp = _es.enter_context(tc.tile_pool(name=f"mstate{rep}", bufs=2))
        statp = _es.enter_context(tc.tile_pool(name=f"stats{rep}", bufs=4))
        bnp = _es.enter_context(tc.tile_pool(name=f"bnconst{rep}", bufs=4))
        dramp = _es.enter_context(tc.tile_pool(name=f"dram{rep}", bufs=2, space="DRAM"))

        eps_t = constp.tile([P, 1], F32, tag="eps")
        nc.vector.memset(eps_t[:], EPS)
        ident = constp.tile([P, P], BF16, tag="ident")
        make_identity(nc, ident[:])

        gbe_sb = {}
        for nm in ("q", "k", "v", "p", "f1", "f2"):
            n_ct = CT_H if nm == "f1" else CT_C
            gt = gbep.tile([P, n_ct], F32, tag=f"g_{nm}")
            bt = gbep.tile([P, n_ct], F32, tag=f"b_{nm}")
            nc.sync.dma_start(out=gt[:], in_=tens[f"{nm}_gp"])
            nc.sync.dma_start(out=bt[:], in_=tens[f"{nm}_bp"])
            gbe_sb[nm] = (gt, bt)

        # x, channel-major [C, S] as 4 tiles of [128, S] f32r
        xs = []
        for ct in range(CT_C):
            tx = xp.tile([P, S], F32R, tag="xs")
            nc.sync.dma_start(out=tx[:], in_=xt_d[ct * P:(ct + 1) * P, :])
            xs.append(tx)

        # ---------- helpers ----------
        def bn_affine(gstats, n_ct, g_sl, b_sl):
            """gstats [128, 2*n_ct] = [sums | sumsqs] (global).
            Returns (a_sc, c_sc) [128, 4*n_ct]: per-t-scaled affine."""
            mean = bnp.tile([P, n_ct], F32, tag="mean")
            var = bnp.tile([P, n_ct], F32, tag="var")
            tmpb = bnp.tile([P, n_ct], F32, tag="btmp")
            nc.vector.tensor_scalar(mean[:], gstats[:, 0:n_ct],
                                    1.0 / S_TOT, None, AX.mult)
            nc.vector.tensor_scalar(var[:], gstats[:, n_ct:2 * n_ct],
                                    1.0 / S_TOT, None, AX.mult)
            nc.vector.tensor_mul(tmpb[:], mean[:], mean[:])
            nc.vector.tensor_tensor(out=var[:], in0=var[:], in1=tmpb[:],
                                    op=AX.subtract)
            nc.scalar.activation(var[:], var[:], AF.Sqrt, bias=eps_t[:])
            nc.vector.reciprocal(var[:], var[:])
            a0 = bnp.tile([P, n_ct], F32, tag="a0")
            c0 = bnp.tile([P, n_ct], F32, tag="c0")
            nc.vector.tensor_mul(a0[:], var[:], g_sl)
            nc.vector.tensor_mul(tmpb[:], mean[:], a0[:])
            nc.vector.tensor_tensor(out=c0[:], in0=b_sl, in1=tmpb[:],
                                    op=AX.subtract)
            a_sc = bnp.tile([P, 4 * n_ct], F32, tag="asc")
            c_sc = bnp.tile([P, 4 * n_ct], F32, tag="csc")
            for t in range(T):
                s = float(2.0 ** (t - 1))
                nc.vector.tensor_scalar(a_sc[:, t * n_ct:(t + 1) * n_ct],
                                        a0[:], s, None, AX.mult)
                nc.vector.tensor_scalar(c_sc[:, t * n_ct:(t + 1) * n_ct],
                                        c0[:], s, None, AX.mult)
            return a_sc, c_sc

        def lif_tile(h_t, ct, n_ct, a_sc, c_sc, spk_writer):
            """LIF over the 4 time slabs of h_t [128, S].
            a_sc/c_sc None -> raw input, scale 2^(t-1) (o-lif).
            spk_writer(t, m_ap) emits the spike tensor for slab t."""
            m = mp.tile([P, SLAB], F32, tag="m")
            for t in range(T):
                sl = h_t[:, t * SLAB:(t + 1) * SLAB]
                thr = float(2.0 ** t)
                if a_sc is None:
                    sa = float(2.0 ** (t - 1))
                    if t == 0:
                        nc.vector.tensor_scalar(m[:], sl, sa, None, AX.mult)
                    else:
                        nc.vector.affine_then_add(m[:], sl, m[:], sa, 0.0)
                else:
                    sa = a_sc[:, t * n_ct + ct:t * n_ct + ct + 1]
                    sc = c_sc[:, t * n_ct + ct:t * n_ct + ct + 1]
                    if t == 0:
                        nc.vector.tensor_scalar(m[:], sl, sa, sc,
                                                AX.mult, AX.add)
                    else:
                        nc.vector.affine_then_add(m[:], sl, m[:], sa, sc)
                spk_writer(t, m[:])
                if t < T - 1:
                    nc.vector._custom_dve(TENSOR_MASK, out=m[:], in0=m[:],
                                          in1=m[:], s0=thr, imm2=0.0)

        def allreduce_stats(arin_sb, width):
            ain = dramp.tile([P, width], F32, tag="arin")
            aout = dramp.tile([P, width], F32, tag="arout", addr_space="Shared")
            nc.sync.dma_start(out=ain[:], in_=arin_sb[:])
            if NO_COLL:
                nc.sync.dma_start(out=aout[:], in_=ain[:])
            else:
                nc.gpsimd.collective_compute(
                    "AllReduce", AX.add, replica_groups=RG,
                    ins=[ain.opt()], outs=[aout.opt()])
            g = statp.tile([P, width], F32, tag="gstats")
            nc.sync.dma_start(out=g[:], in_=aout[:])
            return g

        def mm_layer(n_ct, matmuls_fn, psum_pool, h_pool, descale=1.0):
            """For each output-channel tile ct: run matmuls_fn(ct, ts, ps_slab)
            filling a [P, 4*SLAB] PSUM tile, then one Copy (h + sum accum)
            and one in-place Square (sumsq accum). Returns (h_tiles, arin)."""
            arin = statp.tile([P, 2 * n_ct], F32, tag="arin_sb")
            h_tiles = []
            for ct in range(n_ct):
                ps = psum_pool.tile([P, 4 * SLAB], F32, tag="ps")
                for ts in range(T):
                    matmuls_fn(ct, ts, ps[:, ts * SLAB:(ts + 1) * SLAB])
                h_t = h_pool.tile([P, S], F32, tag="h")
                h_tiles.append(h_t)
                nc.scalar.activation(
                    h_t[:], ps[:], AF.Copy, scale=descale,
                    accum_out=arin[:, ct:ct + 1])
                nc.scalar.activation(
                    ps[:], ps[:], AF.Square, scale=descale,
                    accum_out=arin[:, n_ct + ct:n_ct + ct + 1])
            return h_tiles, arin

        # =======================================================
        # Phases 1-4: qkv (1-pass f32r) / attention / o-lif / p
        # =======================================================
        with ExitStack() as _es1:
            wqp = _es1.enter_context(tc.tile_pool(name=f"wqkv{rep}", bufs=4))
            hp = _es1.enter_context(tc.tile_pool(name=f"hpool{rep}", bufs=6))
            spkp = _es1.enter_context(tc.tile_pool(name=f"spkbf{rep}", bufs=12))
            opairp = _es1.enter_context(tc.tile_pool(name=f"opair{rep}", bufs=2))

            spk_qkv = {}
            with tc.tile_pool(name=f"psqkv{rep}", bufs=2, space="PSUM") as psmm:
                for nm in ("q", "k", "v"):
                    wsb = []
                    for kc in range(CT_C):
                        w_sb = wqp.tile([P, C], F32R, tag="wq")
                        nc.sync.dma_start(
                            out=w_sb[:], in_=w_lin[nm][kc * P:(kc + 1) * P, :])
                        wsb.append(w_sb)

                    def mm_qkv(ct, ts, ps_slab, wsb=wsb):
                        for kc in range(CT_C):
                            nc.tensor.matmul(
                                ps_slab,
                                wsb[kc][:, ct * P:(ct + 1) * P],
                                xs[kc][:, ts * SLAB:(ts + 1) * SLAB],
                                start=(kc == 0), stop=(kc == CT_C - 1))

                    h_tiles, arin = mm_layer(CT_C, mm_qkv, psmm, hp)
                    gst = allreduce_stats(arin, 2 * CT_C)
                    g_t, b_t = gbe_sb[nm]
                    a_sc, c_sc = bn_affine(gst, CT_C, g_t[:], b_t[:])
                    tiles = []
                    for ct in range(CT_C):
                        s_t = spkp.tile([P, S], BF16, tag="spk")
                        tiles.append(s_t)

                        def wr(t, m_ap, s_t=s_t):
                            nc.vector.tensor_scalar(
                                s_t[:, t * SLAB:(t + 1) * SLAB], m_ap,
                                float(2.0 ** t), None, AX.is_gt)

                        lif_tile(h_tiles[ct], ct, CT_C, a_sc, c_sc, wr)
                    spk_qkv[nm] = tiles

            # ---- attention (o_tiles f32; spikes to fp8 DoubleRow pairs) ----
            o_tiles = [hp.tile([P, S], F32, tag="h", name="o_t")
                       for _ in range(CT_C)]
            with ExitStack() as _es2:
                atp = _es2.enter_context(tc.tile_pool(name=f"attn{rep}", bufs=3))
                kvp = _es2.enter_context(tc.tile_pool(name=f"kvp{rep}", bufs=4))
                pst = _es2.enter_context(tc.tile_pool(name=f"pst{rep}", bufs=2, space="PSUM"))
                pskv = _es2.enter_context(tc.tile_pool(name=f"pskv{rep}", bufs=2, space="PSUM"))
                pso = _es2.enter_context(tc.tile_pool(name=f"pso{rep}", bufs=2, space="PSUM"))
                for tb in range(T * BL):
                    base = tb * N
                    kT = [atp.tile([P, C], BF16, tag="kT", name="kT")
                          for _ in range(2)]
                    vT = [atp.tile([P, C], BF16, tag="vT", name="vT")
                          for _ in range(2)]
                    for ct in range(CT_C):
                        for hf in range(2):
                            for src, dst in ((spk_qkv["k"][ct], kT[hf]),
                                             (spk_qkv["v"][ct], vT[hf])):
                                pt = pst.tile([P, P], BF16, tag="pt")
                                nc.tensor.transpose(
                                    pt[:],
                                    src[:, base + hf * P:base + (hf + 1) * P],
                                    ident[:])
                                nc.scalar.copy(
                                    dst[:, ct * P:(ct + 1) * P], pt[:])
                    for ct in range(CT_C):
                        po = pso.tile([P, N], F32, tag="po")
                        for hf in range(2):
                            hd = ct * 2 + hf
                            cols = slice(hd * 64, (hd + 1) * 64)
                            prt = slice(hf * 64, (hf + 1) * 64)
                            pkv = pskv.tile([P, 64], F32, tag="pkv")
                            nc.tensor.matmul(pkv[prt, :], kT[0][:, cols],
                                             vT[0][:, cols],
                                             start=True, stop=False)
                            nc.tensor.matmul(pkv[prt, :], kT[1][:, cols],
                                             vT[1][:, cols],
                                             start=False, stop=True)
                            kv_sb = kvp.tile([P, 64], BF16, tag="kv")
                            nc.scalar.mul(kv_sb[prt, :], pkv[prt, :], SCALE)
                            nc.tensor.matmul(
                                po[prt, :], kv_sb[prt, :],
                                spk_qkv["q"][ct][prt, base:base + N],
                                start=True, stop=True)
                        nc.scalar.copy(
                            o_tiles[ct][:, base:base + N], po[:])

            # ---- o-lif: spikes -> fp8 pair-packed tiles ----
            o_pair = [opairp.tile([P, 2 * S], FP8, tag="opair", name="o_pair")
                      for _ in range(CT_C // 2)]
            for ct in range(CT_C):
                g, i = ct // 2, ct % 2

                def wr_o(t, m_ap, g=g, i=i):
                    nc.vector.tensor_scalar(
                        o_pair[g][:, i * S + t * SLAB:i * S + (t + 1) * SLAB],
                        m_ap, float(2.0 ** t), None, AX.is_gt)

                lif_tile(o_tiles[ct], ct, CT_C, None, None, wr_o)

            # ---- p projection: fp8 DoubleRow, 2-pass weight split ----
            with tc.tile_pool(name=f"psp{rep}", bufs=2, space="PSUM") as psp:
                wp_sb = {}
                for wi, key in ((0, "w_pa"), (1, "w_pb")):
                    for g in range(2):
                        w_sb = wqp.tile([P, 2 * C], FP8, tag="wp8")
                        nc.sync.dma_start(
                            out=w_sb[:], in_=tens[key][g * P:(g + 1) * P, :])
                        wp_sb[(wi, g)] = w_sb

                def mm_p(ct, ts, ps_slab):
                    first = True
                    for wi in range(2):
                        for g in range(2):
                            w3 = wp_sb[(wi, g)][:].rearrange(
                                "p (i m) -> p i m", i=2)
                            s3 = o_pair[g][:].rearrange(
                                "p (i s) -> p i s", i=2)
                            nc.tensor.matmul(
                                ps_slab,
                                w3[:, :, ct * P:(ct + 1) * P],
                                s3[:, :, ts * SLAB:(ts + 1) * SLAB],
                                start=first, stop=(wi == 1 and g == 1),
                                perf_mode=DR)
                            first = False

                hp_tiles, arin_p = mm_layer(CT_C, mm_p, psp, hp, descale=1.0 / WSC)
                gst_p = allreduce_stats(arin_p, 2 * CT_C)
                g_t, b_t = gbe_sb["p"]
                a_sc, c_sc = bn_affine(gst_p, CT_C, g_t[:], b_t[:])
                spk_p = []
                for ct in range(CT_C):
                    s_t = spkp.tile([P, S], BF16, tag="spk")
                    spk_p.append(s_t)

                    def wr_p(t, m_ap, s_t=s_t):
                        nc.vector.tensor_scalar(
                            s_t[:, t * SLAB:(t + 1) * SLAB], m_ap,
                            float(2.0 ** t), None, AX.is_gt)

                    lif_tile(hp_tiles[ct], ct, CT_C, a_sc, c_sc, wr_p)

            # ---- x_res = x + spk_p, in place in the x tiles ----
            xres = xs
            for ct in range(CT_C):
                nc.vector.tensor_tensor(out=xs[ct][:], in0=xs[ct][:],
                                        in1=spk_p[ct][:], op=AX.add)

        # =======================================================
        # Phase 6: f1 (2-pass f32r weight split), AllReduce per half
        # =======================================================
        with ExitStack() as _es3:
            sp8p = _es3.enter_context(tc.tile_pool(name=f"sp8{rep}", bufs=8))
            hp2 = _es3.enter_context(tc.tile_pool(name=f"hpool2{rep}", bufs=8))
            spk_f1p = [sp8p.tile([P, 2 * S], FP8, tag="spk8", name="spk_f1p")
                       for _ in range(CT_H // 2)]
            with ExitStack() as _es3b:
                wsp = _es3b.enter_context(tc.tile_pool(name=f"wsm{rep}", bufs=9))
                psm2 = _es3b.enter_context(
                    tc.tile_pool(name=f"psf1{rep}", bufs=2, space="PSUM"))
                assert F1_PASSES in (1, 2)
                for half in range(2):
                    wt = {}
                    for wi, w_d in ((0, tens["w_f1h"]),
                                    (1, tens["w_f1l"]))[:F1_PASSES]:
                        for kc in range(CT_C):
                            w_sb = wsp.tile([P, 8 * P], F32R, tag="wf1")
                            nc.sync.dma_start(
                                out=w_sb[:],
                                in_=w_d[kc * P:(kc + 1) * P,
                                        half * 8 * P:(half + 1) * 8 * P])
                            for cl in range(8):
                                wt[(wi, kc, cl)] = w_sb[:, cl * P:(cl + 1) * P]

                    def mm_f1(cl, ts, ps_slab, wt=wt):
                        first = True
                        for wi in range(F1_PASSES):
                            for kc in range(CT_C):
                                nc.tensor.matmul(
                                    ps_slab, wt[(wi, kc, cl)],
                                    xres[kc][:, ts * SLAB:(ts + 1) * SLAB],
                                    start=first,
                                    stop=(wi == F1_PASSES - 1 and
                                          kc == CT_C - 1))
                                first = False

                    h_tiles, arin_f = mm_layer(8, mm_f1, psm2, hp2)
                    gst_f = allreduce_stats(arin_f, 16)
                    g_t, b_t = gbe_sb["f1"]
                    a_sc, c_sc = bn_affine(
                        gst_f, 8, g_t[:, half * 8:half * 8 + 8],
                        b_t[:, half * 8:half * 8 + 8])
                    for cl in range(8):
                        ct = half * 8 + cl
                        g, i = ct // 2, ct % 2

                        def wr_f1(t, m_ap, g=g, i=i):
                            nc.vector.tensor_scalar(
                                spk_f1p[g][:, i * S + t * SLAB:
                                           i * S + (t + 1) * SLAB],
                                m_ap, float(2.0 ** t), None, AX.is_gt)

                        lif_tile(h_tiles[cl], cl, 8, a_sc, c_sc, wr_f1)

            # =====================================================
            # Phase 7: f2: fp8 DoubleRow, 3-way weight split
            # =====================================================
            with ExitStack() as _es4:
                wf2p = _es4.enter_context(tc.tile_pool(name=f"wf2p{rep}", bufs=24))
                psm3 = _es4.enter_context(
                    tc.tile_pool(name=f"psf2{rep}", bufs=2, space="PSUM"))
                wf2_sb = {}
                for wi, key in ((0, "w_f2a"), (1, "w_f2b"), (2, "w_f2c")):
                    for g in range(CT_H // 2):
                        w_sb = wf2p.tile([P, 2 * C], FP8, tag="wf2")
                        nc.sync.dma_start(
                            out=w_sb[:], in_=tens[key][g * P:(g + 1) * P, :])
                        wf2_sb[(wi, g)] = w_sb

                def mm_f2(ct, ts, ps_slab):
                    first = True
                    for wi in range(3):
                        for g in range(CT_H // 2):
                            w3 = wf2_sb[(wi, g)][:].rearrange(
                                "p (i m) -> p i m", i=2)
                            s3 = spk_f1p[g][:].rearrange(
                                "p (i s) -> p i s", i=2)
                            nc.tensor.matmul(
                                ps_slab,
                                w3[:, :, ct * P:(ct + 1) * P],
                                s3[:, :, ts * SLAB:(ts + 1) * SLAB],
                                start=first,
                                stop=(wi == 2 and g == CT_H // 2 - 1),
                                perf_mode=DR)
                            first = False

                h2_tiles, arin_2 = mm_layer(CT_C, mm_f2, psm3, hp2,
                                            descale=1.0 / WSC)
                gst_2 = allreduce_stats(arin_2, 2 * CT_C)
                g_t, b_t = gbe_sb["f2"]
                a_sc, c_sc = bn_affine(gst_2, CT_C, g_t[:], b_t[:])
                for ct in range(CT_C):
                    h_t = h2_tiles[ct]

                    def wr_f2(t, m_ap, h_t=h_t):
                        nc.vector.tensor_scalar(
                            h_t[:, t * SLAB:(t + 1) * SLAB], m_ap,
                            float(2.0 ** t), None, AX.is_gt)

                    lif_tile(h_t, ct, CT_C, a_sc, c_sc, wr_f2)

                # out = x_res + spk_f2
                for ct in range(CT_C):
                    tmp = hp2.tile([P, S], F32, tag="h", name="outadd")
                    nc.vector.tensor_tensor(out=tmp[:], in0=xres[ct][:],
                                            in1=h2_tiles[ct][:], op=AX.add)
                    nc.sync.dma_start(
                        out=tens["outT"][ct * P:(ct + 1) * P, :], in_=tmp[:])


def _build(replicas=1):
    nc = bacc.Bacc("TRN2", target_bir_lowering=False, debug=False,
                   num_devices=NCORES)

    def dt_in(name, shape, dtype):
        return nc.dram_tensor(name, list(shape), dtype,
                              kind="ExternalInput").ap()

    tens = {"xt": dt_in("xt", (C, S), F32R)}
    for nm in ("q", "k", "v"):
        tens["w_" + nm] = dt_in("w_" + nm, (C, C), F32R)
    tens["w_pa"] = dt_in("w_pa", (C // 2, 2 * C), FP8)
    tens["w_pb"] = dt_in("w_pb", (C // 2, 2 * C), FP8)
    tens["w_f1h"] = dt_in("w_f1h", (C, HID), F32R)
    tens["w_f1l"] = dt_in("w_f1l", (C, HID), F32R)
    tens["w_f2a"] = dt_in("w_f2a", (HID // 2, 2 * C), FP8)
    tens["w_f2b"] = dt_in("w_f2b", (HID // 2, 2 * C), FP8)
    tens["w_f2c"] = dt_in("w_f2c", (HID // 2, 2 * C), FP8)
    for nm in ("q", "k", "v", "p", "f1", "f2"):
        n_ct = CT_H if nm == "f1" else CT_C
        tens[f"{nm}_gp"] = dt_in(f"{nm}_gp", (P, n_ct), F32)
        tens[f"{nm}_bp"] = dt_in(f"{nm}_bp", (P, n_ct), F32)
    tens["outT"] = nc.dram_tensor("outT", [C, S], F32,
                                  kind="ExternalOutput").ap()

    with tile.TileContext(nc) as tc:
        for rep in range(replicas):
            _emit(nc, tc, tens, rep)

    nc.compile()
    return nc


def _get_nc(replicas=1):
    key = ("nc", replicas)
    if key not in _CACHE:
        _CACHE[key] = _build(replicas)
    return _CACHE[key]


def _make_in_maps(inputs):
    x = np.asarray(inputs["x"], np.float32)
    base = {}
    for nm in ("q", "k", "v"):
        base[f"w_{nm}"] = _round_mant(np.asarray(inputs[f"{nm}_W"],
                                                 np.float32))
    base["w_pa"], base["w_pb"] = _fp8_dr_pack(
        np.asarray(inputs["p_W"], np.float32), 2)
    base["w_f1h"], base["w_f1l"] = _f32r_split(np.asarray(inputs["f1_W"],
                                                          np.float32))
    base["w_f2a"], base["w_f2b"], base["w_f2c"] = _fp8_dr_pack(
        np.asarray(inputs["f2_W"], np.float32), 3)
    for nm in ("q", "k", "v", "p", "f1", "f2"):
        n_ct = CT_H if nm == "f1" else CT_C
        base[f"{nm}_gp"] = _pack_ch(inputs[f"{nm}_g"], n_ct)
        base[f"{nm}_bp"] = _pack_ch(inputs[f"{nm}_be"], n_ct)
    in_maps = []
    for c in range(NCORES):
        xsl = x[:, c * BL:(c + 1) * BL].reshape(S, C)
        m = dict(base)
        m["xt"] = _round_mant(np.ascontiguousarray(xsl.T))
        in_maps.append(m)
    return in_maps


def kernel(**inputs):
    in_maps = _make_in_maps(inputs)
    nc = _get_nc()
    res = bass_utils.run_bass_kernel_spmd(nc, in_maps,
                                          core_ids=list(range(NCORES)))
    _CACHE["last_results"] = res

    out = np.empty((T, B, N, C), np.float32)
    for c in range(NCORES):
        oc = np.asarray(res.results[c]["outT"])   # [C, S]
        out[:, c * BL:(c + 1) * BL] = oc.T.reshape(T, BL, N, C)
    return out
